# revision 8
# baseline (speedup 1.0000x reference)
"""DoRA linear layer (nn_DoRALinear) on 8 Trainium2 NeuronCores.

Tensor-parallel over out_features (column parallel): each core owns 512 of the
4096 output columns — its shard of base_weight, lora_B and magnitude_vec —
while x and lora_A are replicated. No cross-core communication is needed.

Per-core program (all matmuls in fp16 — 1 PE cycle/row; fp32 runs at 1/4 rate
and fp32r at 1/2 rate on TRN2 — with fp32 PSUM accumulation):
  phase A1: W'T[k,o] = wt[k,o] + (lora_A^T @ lora_B^T)[k,o]
            (PE matmul K=16 + DVE add, 32 resident [128,512] k-tiles)
  phase B:  y[t,o] = (sum_k x^T[k,t] * W'T[k,o]) * scale[o]
            64 token-tiles x 32 k-tile accumulating matmuls into PSUM,
            DVE epilogue multiplies by scale while evicting PSUM->SBUF
  phase A2 (off critical path): norm2 = ones^T @ (W'T)^2 via PE partition
            reduction; scale = magnitude / sqrt(norm2), broadcast to 128
            partitions once via GPSIMD.

x^T is pre-packed on the host as [group, k-tile, 128, 512] so every x DMA is
one linear 128KB HBM read.
"""
import numpy as np

import concourse.bass as bass
import concourse.bass_isa as bass_isa
import concourse.mybir as mybir
import concourse.tile as tile
from concourse import bacc
from concourse.bass_utils import run_bass_kernel_spmd

P = 128
f32 = mybir.dt.float32
DT = mybir.dt.float16
NP_DT = np.float16

IN_F = 4096          # contraction dim K
OUT_F = 4096
O = 512              # out-features per core
KT = IN_F // P       # 32 k-tiles
N_CORES = 8
M_TOKENS = 8192
T_GROUP = 512


def _build(m_tokens=M_TOKENS, t_group=T_GROUP):
    n_groups = m_tokens // t_group
    groups = [t_group] * n_groups

    nc = bacc.Bacc(None, target_bir_lowering=False, debug=False)

    xt_d = nc.dram_tensor(
        "xt", [n_groups, KT, P, t_group], DT, kind="ExternalInput"
    )
    wt_d = nc.dram_tensor("wt", [IN_F, O], DT, kind="ExternalInput")
    a_d = nc.dram_tensor("a", [16, IN_F], DT, kind="ExternalInput")
    bt_d = nc.dram_tensor("bt", [16, O], DT, kind="ExternalInput")
    mg_d = nc.dram_tensor("mg", [1, O], f32, kind="ExternalInput")
    ones_d = nc.dram_tensor("ones", [P, 1], DT, kind="ExternalInput")
    y_d = nc.dram_tensor("y", [m_tokens, O], f32, kind="ExternalOutput")

    with tile.TileContext(nc) as tc:
        with (
            tc.tile_pool(name="const", bufs=1) as const_pool,
            tc.tile_pool(name="wpt", bufs=KT) as wpt_pool,
            tc.tile_pool(name="wtin", bufs=3) as wtin_pool,
            tc.tile_pool(name="sq", bufs=KT) as sq_pool,
            tc.tile_pool(name="xg", bufs=2 * KT) as xg_pool,
            tc.tile_pool(name="yout", bufs=6) as y_pool,
            tc.tile_pool(name="ps_ba", bufs=2, space="PSUM") as ps_ba_pool,
            tc.tile_pool(name="ps_y", bufs=6, space="PSUM") as ps_y_pool,
        ):
            a_sb = const_pool.tile([16, IN_F], DT, name="a_sb")
            bt_sb = const_pool.tile([16, O], DT, name="bt_sb")
            m_sb = const_pool.tile([1, O], f32, name="m_sb")
            ones_sb = const_pool.tile([P, 1], DT, name="ones_sb")
            nc.sync.dma_start(a_sb[:], a_d[:])
            nc.sync.dma_start(bt_sb[:], bt_d[:])
            nc.sync.dma_start(m_sb[:], mg_d[:])
            nc.sync.dma_start(ones_sb[:], ones_d[:])

            # ---- phase A1: adapted weight tiles (gates the main GEMM) ----
            # wt and group-0 x DMAs alternate on the same queue so wpt[kt]
            # and xg0[kt] arrive in lockstep — the first token-tile's matmuls
            # stream right behind the DMA pair stream
            wpt = []
            xg0 = []
            for kt in range(KT):
                pba = ps_ba_pool.tile([P, O], f32, name="pba", tag="pba")
                nc.tensor.matmul(
                    pba[:], a_sb[:, kt * P:(kt + 1) * P], bt_sb[:],
                    start=True, stop=True,
                )
                wtin = wtin_pool.tile([P, O], DT, name="wtin", tag="wtin")
                nc.sync.dma_start(wtin[:], wt_d[kt * P:(kt + 1) * P, :])
                xt_t = xg_pool.tile([P, t_group], DT, name=f"xg{kt}", tag="xg")
                nc.sync.dma_start(xt_t[:], xt_d[0, kt])
                xg0.append(xt_t)
                wp = wpt_pool.tile([P, O], DT, name=f"wpt{kt}", tag="wpt")
                nc.vector.tensor_add(wp[:], pba[:], wtin[:])
                wpt.append(wp)

            def emit_group_mms(g_i, g_sz, xg=None):
                subs = g_sz // P
                if xg is None:
                    xg = []
                    for kt in range(KT):
                        xt_t = xg_pool.tile(
                            [P, t_group], DT, name=f"xg{kt}", tag="xg"
                        )
                        nc.sync.dma_start(xt_t[:], xt_d[g_i, kt])
                        xg.append(xt_t)
                pys = [
                    ps_y_pool.tile([P, O], f32, name="py", tag="py")
                    for _ in range(subs)
                ]
                for sub in range(subs):
                    for kt in range(KT):
                        nc.tensor.matmul(
                            pys[sub][:],
                            xg[kt][:, sub * P:(sub + 1) * P], wpt[kt][:],
                            start=(kt == 0), stop=(kt == KT - 1),
                        )
                return pys

            def emit_epilogues(t0, pys):
                for sub, py in enumerate(pys):
                    yt = y_pool.tile([P, O], f32, name="yt", tag="yt")
                    nc.vector.tensor_mul(yt[:], py[:], scale_t[:])
                    r0 = t0 + sub * P
                    nc.scalar.dma_start(y_d[r0:r0 + P, :], yt[:])

            # ---- phase B group 0 matmuls (PE works while norms compute) ----
            pys0 = emit_group_mms(0, groups[0], xg=xg0)

            # ---- phase A2: column norms + scale (entirely off the PE: ACT
            # squares, DVE accumulates, GPSIMD reduces across partitions;
            # only the first epilogue waits on scale_t, buffered by ps_y) ----
            sqsum = const_pool.tile([P, O], f32, name="sqsum")
            for kt in range(KT):
                sq = sq_pool.tile([P, O], f32, name="sq", tag="sq")
                nc.scalar.square(sq[:], wpt[kt][:])
                if kt == 0:
                    nc.vector.tensor_copy(sqsum[:], sq[:])
                else:
                    nc.vector.tensor_add(sqsum[:], sqsum[:], sq[:])
            nsum = const_pool.tile([P, O], f32, name="nsum")
            nc.gpsimd.partition_all_reduce(
                nsum[:], sqsum[:], channels=P, reduce_op=bass_isa.ReduceOp.add
            )
            nrm = const_pool.tile([1, O], f32, name="nrm")
            nc.scalar.sqrt(nrm[:], nsum[:1, :])
            inv = const_pool.tile([1, O], f32, name="inv")
            nc.vector.reciprocal(inv[:], nrm[:])
            srow = const_pool.tile([1, O], f32, name="srow")
            nc.vector.tensor_mul(srow[:], inv[:], m_sb[:])
            scale_t = const_pool.tile([P, O], f32, name="scale_t")
            nc.gpsimd.partition_broadcast(scale_t[:], srow[:])

            emit_epilogues(0, pys0)

            # ---- phase B: remaining groups ----
            t0 = groups[0]
            for g_i, g_sz in enumerate(groups[1:], start=1):
                pys = emit_group_mms(g_i, g_sz)
                emit_epilogues(t0, pys)
                t0 += g_sz

    nc.compile()
    return nc


_NC_CACHE = {}


def kernel(x, base_weight, lora_A, lora_B, magnitude_vec):
    x = np.asarray(x)
    base_weight = np.asarray(base_weight)
    lora_A = np.asarray(lora_A)
    lora_B = np.asarray(lora_B)
    magnitude_vec = np.asarray(magnitude_vec)

    out_shape = (*x.shape[:-1], base_weight.shape[0])
    m_tokens = x.size // IN_F

    if m_tokens not in _NC_CACHE:
        _NC_CACHE[m_tokens] = _build(m_tokens)
    nc = _NC_CACHE[m_tokens]

    n_groups = m_tokens // T_GROUP
    xt = np.ascontiguousarray(x.reshape(m_tokens, IN_F).T).astype(NP_DT)
    xt = np.ascontiguousarray(
        xt.reshape(KT, P, n_groups, T_GROUP).transpose(2, 0, 1, 3)
    )
    a_c = lora_A.astype(NP_DT)
    b_c = lora_B.astype(NP_DT)
    ones = np.ones((P, 1), dtype=NP_DT)

    in_maps = []
    for c in range(N_CORES):
        sl = slice(c * O, (c + 1) * O)
        in_maps.append({
            "xt": xt,
            "wt": np.ascontiguousarray(base_weight[sl].T).astype(NP_DT),
            "a": a_c,
            "bt": np.ascontiguousarray(b_c[sl].T),
            "mg": np.ascontiguousarray(
                magnitude_vec[sl][None, :].astype(np.float32)
            ),
            "ones": ones,
        })

    res = run_bass_kernel_spmd(nc, in_maps, core_ids=list(range(N_CORES)))
    y = np.concatenate(
        [res.results[c]["y"] for c in range(N_CORES)], axis=1
    )
    return y.reshape(out_shape).astype(np.float32)


# revision 9
# speedup vs baseline: 1.0022x; 1.0022x over previous
"""DoRA linear layer (nn_DoRALinear) on 8 Trainium2 NeuronCores.

Tensor-parallel over out_features (column parallel): each core owns 512 of the
4096 output columns — its shard of base_weight, lora_B and magnitude_vec —
while x and lora_A are replicated. No cross-core communication is needed.

Per-core program (all matmuls in fp16 — 1 PE cycle/row; fp32 runs at 1/4 rate
and fp32r at 1/2 rate on TRN2 — with fp32 PSUM accumulation):
  phase A1: W'T[k,o] = wt[k,o] + (lora_A^T @ lora_B^T)[k,o]
            (PE matmul K=16 + DVE add, 32 resident [128,512] k-tiles)
  phase B:  y[t,o] = (sum_k x^T[k,t] * W'T[k,o]) * scale[o]
            64 token-tiles x 32 k-tile accumulating matmuls into PSUM,
            DVE epilogue multiplies by scale while evicting PSUM->SBUF
  phase A2 (off critical path): norm2 = ones^T @ (W'T)^2 via PE partition
            reduction; scale = magnitude / sqrt(norm2), broadcast to 128
            partitions once via GPSIMD.

x^T is pre-packed on the host as [group, k-tile, 128, 512] so every x DMA is
one linear 128KB HBM read.
"""
import numpy as np

import concourse.bass as bass
import concourse.bass_isa as bass_isa
import concourse.mybir as mybir
import concourse.tile as tile
from concourse import bacc
from concourse.bass_utils import run_bass_kernel_spmd

P = 128
f32 = mybir.dt.float32
DT = mybir.dt.float16
NP_DT = np.float16

IN_F = 4096          # contraction dim K
OUT_F = 4096
O = 512              # out-features per core
KT = IN_F // P       # 32 k-tiles
N_CORES = 8
M_TOKENS = 8192
T_GROUP = 512


def _build(m_tokens=M_TOKENS, t_group=T_GROUP):
    n_groups = m_tokens // t_group
    groups = [t_group] * n_groups

    nc = bacc.Bacc(None, target_bir_lowering=False, debug=False)

    xt_d = nc.dram_tensor(
        "xt", [n_groups, KT, P, t_group], DT, kind="ExternalInput"
    )
    wt_d = nc.dram_tensor("wt", [IN_F, O], DT, kind="ExternalInput")
    a_d = nc.dram_tensor("a", [16, IN_F], DT, kind="ExternalInput")
    bt_d = nc.dram_tensor("bt", [16, O], DT, kind="ExternalInput")
    mg_d = nc.dram_tensor("mg", [1, O], f32, kind="ExternalInput")
    ones_d = nc.dram_tensor("ones", [P, 1], DT, kind="ExternalInput")
    y_d = nc.dram_tensor("y", [m_tokens, O], f32, kind="ExternalOutput")

    with tile.TileContext(nc) as tc:
        with (
            tc.tile_pool(name="const", bufs=1) as const_pool,
            tc.tile_pool(name="wpt", bufs=KT) as wpt_pool,
            tc.tile_pool(name="wtin", bufs=3) as wtin_pool,
            tc.tile_pool(name="sq", bufs=KT) as sq_pool,
            tc.tile_pool(name="xg", bufs=2 * KT) as xg_pool,
            tc.tile_pool(name="yout", bufs=6) as y_pool,
            tc.tile_pool(name="ps_ba", bufs=2, space="PSUM") as ps_ba_pool,
            tc.tile_pool(name="ps_y", bufs=6, space="PSUM") as ps_y_pool,
        ):
            a_sb = const_pool.tile([16, IN_F], DT, name="a_sb")
            bt_sb = const_pool.tile([16, O], DT, name="bt_sb")
            m_sb = const_pool.tile([1, O], f32, name="m_sb")
            ones_sb = const_pool.tile([P, 1], DT, name="ones_sb")
            nc.sync.dma_start(a_sb[:], a_d[:])
            nc.sync.dma_start(bt_sb[:], bt_d[:])
            nc.sync.dma_start(m_sb[:], mg_d[:])
            nc.sync.dma_start(ones_sb[:], ones_d[:])

            # ---- phase A1: adapted weight tiles (gates the main GEMM) ----
            # wt and group-0 x DMAs alternate on the same queue so wpt[kt]
            # and xg0[kt] arrive in lockstep — the first token-tile's matmuls
            # stream right behind the DMA pair stream
            wpt = []
            xg0 = []
            for kt in range(KT):
                pba = ps_ba_pool.tile([P, O], f32, name="pba", tag="pba")
                nc.tensor.matmul(
                    pba[:], a_sb[:, kt * P:(kt + 1) * P], bt_sb[:],
                    start=True, stop=True,
                )
                wtin = wtin_pool.tile([P, O], DT, name="wtin", tag="wtin")
                nc.scalar.dma_start(wtin[:], wt_d[kt * P:(kt + 1) * P, :])
                xt_t = xg_pool.tile([P, t_group], DT, name=f"xg{kt}", tag="xg")
                nc.sync.dma_start(xt_t[:], xt_d[0, kt])
                xg0.append(xt_t)
                wp = wpt_pool.tile([P, O], DT, name=f"wpt{kt}", tag="wpt")
                nc.vector.tensor_add(wp[:], pba[:], wtin[:])
                wpt.append(wp)

            def emit_group_mms(g_i, g_sz, xg=None):
                subs = g_sz // P
                if xg is None:
                    xg = []
                    for kt in range(KT):
                        xt_t = xg_pool.tile(
                            [P, t_group], DT, name=f"xg{kt}", tag="xg"
                        )
                        nc.sync.dma_start(xt_t[:], xt_d[g_i, kt])
                        xg.append(xt_t)
                pys = [
                    ps_y_pool.tile([P, O], f32, name="py", tag="py")
                    for _ in range(subs)
                ]
                for sub in range(subs):
                    for kt in range(KT):
                        nc.tensor.matmul(
                            pys[sub][:],
                            xg[kt][:, sub * P:(sub + 1) * P], wpt[kt][:],
                            start=(kt == 0), stop=(kt == KT - 1),
                        )
                return pys

            def emit_epilogues(t0, pys):
                for sub, py in enumerate(pys):
                    yt = y_pool.tile([P, O], f32, name="yt", tag="yt")
                    nc.vector.tensor_mul(yt[:], py[:], scale_t[:])
                    r0 = t0 + sub * P
                    nc.scalar.dma_start(y_d[r0:r0 + P, :], yt[:])

            # ---- phase B group 0 matmuls (PE works while norms compute) ----
            pys0 = emit_group_mms(0, groups[0], xg=xg0)

            # ---- phase A2: column norms + scale (entirely off the PE: ACT
            # squares, DVE accumulates, GPSIMD reduces across partitions;
            # only the first epilogue waits on scale_t, buffered by ps_y) ----
            sqsum = const_pool.tile([P, O], f32, name="sqsum")
            for kt in range(KT):
                sq = sq_pool.tile([P, O], f32, name="sq", tag="sq")
                nc.scalar.square(sq[:], wpt[kt][:])
                if kt == 0:
                    nc.vector.tensor_copy(sqsum[:], sq[:])
                else:
                    nc.vector.tensor_add(sqsum[:], sqsum[:], sq[:])
            nsum = const_pool.tile([P, O], f32, name="nsum")
            nc.gpsimd.partition_all_reduce(
                nsum[:], sqsum[:], channels=P, reduce_op=bass_isa.ReduceOp.add
            )
            nrm = const_pool.tile([1, O], f32, name="nrm")
            nc.scalar.sqrt(nrm[:], nsum[:1, :])
            inv = const_pool.tile([1, O], f32, name="inv")
            nc.vector.reciprocal(inv[:], nrm[:])
            srow = const_pool.tile([1, O], f32, name="srow")
            nc.vector.tensor_mul(srow[:], inv[:], m_sb[:])
            scale_t = const_pool.tile([P, O], f32, name="scale_t")
            nc.gpsimd.partition_broadcast(scale_t[:], srow[:])

            emit_epilogues(0, pys0)

            # ---- phase B: remaining groups ----
            t0 = groups[0]
            for g_i, g_sz in enumerate(groups[1:], start=1):
                pys = emit_group_mms(g_i, g_sz)
                emit_epilogues(t0, pys)
                t0 += g_sz

    nc.compile()
    return nc


_NC_CACHE = {}


def kernel(x, base_weight, lora_A, lora_B, magnitude_vec):
    x = np.asarray(x)
    base_weight = np.asarray(base_weight)
    lora_A = np.asarray(lora_A)
    lora_B = np.asarray(lora_B)
    magnitude_vec = np.asarray(magnitude_vec)

    out_shape = (*x.shape[:-1], base_weight.shape[0])
    m_tokens = x.size // IN_F

    if m_tokens not in _NC_CACHE:
        _NC_CACHE[m_tokens] = _build(m_tokens)
    nc = _NC_CACHE[m_tokens]

    n_groups = m_tokens // T_GROUP
    xt = np.ascontiguousarray(x.reshape(m_tokens, IN_F).T).astype(NP_DT)
    xt = np.ascontiguousarray(
        xt.reshape(KT, P, n_groups, T_GROUP).transpose(2, 0, 1, 3)
    )
    a_c = lora_A.astype(NP_DT)
    b_c = lora_B.astype(NP_DT)
    ones = np.ones((P, 1), dtype=NP_DT)

    in_maps = []
    for c in range(N_CORES):
        sl = slice(c * O, (c + 1) * O)
        in_maps.append({
            "xt": xt,
            "wt": np.ascontiguousarray(base_weight[sl].T).astype(NP_DT),
            "a": a_c,
            "bt": np.ascontiguousarray(b_c[sl].T),
            "mg": np.ascontiguousarray(
                magnitude_vec[sl][None, :].astype(np.float32)
            ),
            "ones": ones,
        })

    res = run_bass_kernel_spmd(nc, in_maps, core_ids=list(range(N_CORES)))
    y = np.concatenate(
        [res.results[c]["y"] for c in range(N_CORES)], axis=1
    )
    return y.reshape(out_shape).astype(np.float32)


# revision 10
# speedup vs baseline: 1.0133x; 1.0111x over previous
"""DoRA linear layer (nn_DoRALinear) on 8 Trainium2 NeuronCores.

Tensor-parallel over out_features (column parallel): each core owns 512 of the
4096 output columns — its shard of base_weight, lora_B and magnitude_vec —
while x and lora_A are replicated. No cross-core communication is needed.

Per-core program (all matmuls in fp16 — 1 PE cycle/row; fp32 runs at 1/4 rate
and fp32r at 1/2 rate on TRN2 — with fp32 PSUM accumulation):
  phase A1: W'T[k,o] = wt[k,o] + (lora_A^T @ lora_B^T)[k,o]
            (PE matmul K=16 + DVE add, 32 resident [128,512] k-tiles)
  phase B:  y[t,o] = (sum_k x^T[k,t] * W'T[k,o]) * scale[o]
            64 token-tiles x 32 k-tile accumulating matmuls into PSUM,
            DVE epilogue multiplies by scale while evicting PSUM->SBUF
  phase A2 (off critical path): norm2 = ones^T @ (W'T)^2 via PE partition
            reduction; scale = magnitude / sqrt(norm2), broadcast to 128
            partitions once via GPSIMD.

x^T is pre-packed on the host as [group, k-tile, 128, 512] so every x DMA is
one linear 128KB HBM read.
"""
import numpy as np

import concourse.bass as bass
import concourse.bass_isa as bass_isa
import concourse.mybir as mybir
import concourse.tile as tile
from concourse import bacc
from concourse.bass_utils import run_bass_kernel_spmd

P = 128
f32 = mybir.dt.float32
DT = mybir.dt.float16
NP_DT = np.float16

IN_F = 4096          # contraction dim K
OUT_F = 4096
O = 512              # out-features per core
KT = IN_F // P       # 32 k-tiles
N_CORES = 8
M_TOKENS = 8192
T_GROUP = 512


def _build(m_tokens=M_TOKENS, t_group=T_GROUP):
    n_groups = m_tokens // t_group
    groups = [t_group] * n_groups

    nc = bacc.Bacc(None, target_bir_lowering=False, debug=False)

    xt_d = nc.dram_tensor(
        "xt", [n_groups, KT, P, t_group], DT, kind="ExternalInput"
    )
    wt_d = nc.dram_tensor("wt", [IN_F, O], DT, kind="ExternalInput")
    a_d = nc.dram_tensor("a", [16, IN_F], DT, kind="ExternalInput")
    bt_d = nc.dram_tensor("bt", [16, O], DT, kind="ExternalInput")
    mg_d = nc.dram_tensor("mg", [1, O], f32, kind="ExternalInput")
    ones_d = nc.dram_tensor("ones", [P, 1], DT, kind="ExternalInput")
    y_d = nc.dram_tensor("y", [m_tokens, O], f32, kind="ExternalOutput")

    with tile.TileContext(nc) as tc:
        with (
            tc.tile_pool(name="const", bufs=1) as const_pool,
            tc.tile_pool(name="wpt", bufs=KT) as wpt_pool,
            tc.tile_pool(name="wtin", bufs=3) as wtin_pool,
            tc.tile_pool(name="sq", bufs=KT) as sq_pool,
            tc.tile_pool(name="xg", bufs=2 * KT) as xg_pool,
            tc.tile_pool(name="yout", bufs=6) as y_pool,
            tc.tile_pool(name="ps_ba", bufs=2, space="PSUM") as ps_ba_pool,
            tc.tile_pool(name="ps_y", bufs=6, space="PSUM") as ps_y_pool,
        ):
            a_sb = const_pool.tile([16, IN_F], DT, name="a_sb")
            bt_sb = const_pool.tile([16, O], DT, name="bt_sb")
            m_sb = const_pool.tile([1, O], f32, name="m_sb")
            ones_sb = const_pool.tile([P, 1], DT, name="ones_sb")
            nc.sync.dma_start(a_sb[:], a_d[:])
            nc.sync.dma_start(bt_sb[:], bt_d[:])
            nc.sync.dma_start(m_sb[:], mg_d[:])
            nc.sync.dma_start(ones_sb[:], ones_d[:])

            # ---- phase A1: adapted weight tiles (gates the main GEMM) ----
            # wt and group-0 x DMAs alternate on the same queue so wpt[kt]
            # and xg0[kt] arrive in lockstep — the first token-tile's matmuls
            # stream right behind the DMA pair stream
            wpt = []
            xg0 = []
            for kt in range(KT):
                pba = ps_ba_pool.tile([P, O], f32, name="pba", tag="pba")
                nc.tensor.matmul(
                    pba[:], a_sb[:, kt * P:(kt + 1) * P], bt_sb[:],
                    start=True, stop=True,
                )
                wtin = wtin_pool.tile([P, O], DT, name="wtin", tag="wtin")
                nc.sync.dma_start(wtin[:], wt_d[kt * P:(kt + 1) * P, :])
                xt_t = xg_pool.tile([P, t_group], DT, name=f"xg{kt}", tag="xg")
                nc.sync.dma_start(xt_t[:], xt_d[0, kt])
                xg0.append(xt_t)
                wp = wpt_pool.tile([P, O], DT, name=f"wpt{kt}", tag="wpt")
                nc.vector.tensor_add(wp[:], pba[:], wtin[:])
                wpt.append(wp)

            def emit_group_mms(g_i, g_sz, xg=None):
                subs = g_sz // P
                if xg is None:
                    xg = []
                    for kt in range(KT):
                        xt_t = xg_pool.tile(
                            [P, t_group], DT, name=f"xg{kt}", tag="xg"
                        )
                        nc.sync.dma_start(xt_t[:], xt_d[g_i, kt])
                        xg.append(xt_t)
                pys = [
                    ps_y_pool.tile([P, O], f32, name="py", tag="py")
                    for _ in range(subs)
                ]
                for sub in range(subs):
                    for kt in range(KT):
                        nc.tensor.matmul(
                            pys[sub][:],
                            xg[kt][:, sub * P:(sub + 1) * P], wpt[kt][:],
                            start=(kt == 0), stop=(kt == KT - 1),
                        )
                return pys

            def emit_epilogues(t0, pys):
                for sub, py in enumerate(pys):
                    yt = y_pool.tile([P, O], f32, name="yt", tag="yt")
                    nc.vector.tensor_mul(yt[:], py[:], scale_t[:])
                    r0 = t0 + sub * P
                    nc.scalar.dma_start(y_d[r0:r0 + P, :], yt[:])

            # ---- phase B group 0 matmuls (PE works while norms compute) ----
            pys0 = emit_group_mms(0, groups[0], xg=xg0)

            # ---- phase A2: column norms + scale (entirely off the PE: ACT
            # squares, DVE accumulates, GPSIMD reduces across partitions;
            # only the first epilogue waits on scale_t, buffered by ps_y) ----
            sqsum = const_pool.tile([P, O], f32, name="sqsum")
            for kt in range(KT):
                sq = sq_pool.tile([P, O], f32, name="sq", tag="sq")
                nc.scalar.square(sq[:], wpt[kt][:])
                if kt == 0:
                    nc.vector.tensor_copy(sqsum[:], sq[:])
                else:
                    nc.vector.tensor_add(sqsum[:], sqsum[:], sq[:])
            nsum = const_pool.tile([P, O], f32, name="nsum")
            nc.gpsimd.partition_all_reduce(
                nsum[:], sqsum[:], channels=P, reduce_op=bass_isa.ReduceOp.add
            )
            nrm = const_pool.tile([1, O], f32, name="nrm")
            nc.scalar.sqrt(nrm[:], nsum[:1, :])
            inv = const_pool.tile([1, O], f32, name="inv")
            nc.vector.reciprocal(inv[:], nrm[:])
            srow = const_pool.tile([1, O], f32, name="srow")
            nc.vector.tensor_mul(srow[:], inv[:], m_sb[:])
            scale_t = const_pool.tile([P, O], f32, name="scale_t")
            nc.gpsimd.partition_broadcast(scale_t[:], srow[:])

            emit_epilogues(0, pys0)

            # ---- phase B: remaining groups ----
            t0 = groups[0]
            for g_i, g_sz in enumerate(groups[1:], start=1):
                pys = emit_group_mms(g_i, g_sz)
                emit_epilogues(t0, pys)
                t0 += g_sz

    nc.compile()
    return nc


_NC_CACHE = {}


def kernel(x, base_weight, lora_A, lora_B, magnitude_vec):
    x = np.asarray(x)
    base_weight = np.asarray(base_weight)
    lora_A = np.asarray(lora_A)
    lora_B = np.asarray(lora_B)
    magnitude_vec = np.asarray(magnitude_vec)

    out_shape = (*x.shape[:-1], base_weight.shape[0])
    m_tokens = x.size // IN_F

    if m_tokens not in _NC_CACHE:
        _NC_CACHE[m_tokens] = _build(m_tokens)
    nc = _NC_CACHE[m_tokens]

    n_groups = m_tokens // T_GROUP
    xt = np.ascontiguousarray(x.reshape(m_tokens, IN_F).T).astype(NP_DT)
    xt = np.ascontiguousarray(
        xt.reshape(KT, P, n_groups, T_GROUP).transpose(2, 0, 1, 3)
    )
    a_c = lora_A.astype(NP_DT)
    b_c = lora_B.astype(NP_DT)
    ones = np.ones((P, 1), dtype=NP_DT)

    in_maps = []
    for c in range(N_CORES):
        sl = slice(c * O, (c + 1) * O)
        in_maps.append({
            "xt": xt,
            "wt": np.ascontiguousarray(base_weight[sl].T).astype(NP_DT),
            "a": a_c,
            "bt": np.ascontiguousarray(b_c[sl].T),
            "mg": np.ascontiguousarray(
                magnitude_vec[sl][None, :].astype(np.float32)
            ),
            "ones": ones,
        })

    res = run_bass_kernel_spmd(nc, in_maps, core_ids=list(range(N_CORES)))
    y = np.concatenate(
        [res.results[c]["y"] for c in range(N_CORES)], axis=1
    )
    return y.reshape(out_shape).astype(np.float32)


# revision 11
# speedup vs baseline: 1.0168x; 1.0034x over previous
"""DoRA linear layer (nn_DoRALinear) on 8 Trainium2 NeuronCores.

Tensor-parallel over out_features (column parallel): each core owns 512 of the
4096 output columns — its shard of base_weight, lora_B and magnitude_vec —
while x and lora_A are replicated. No cross-core communication is needed.

Per-core program (all matmuls in fp16 — 1 PE cycle/row; fp32 runs at 1/4 rate
and fp32r at 1/2 rate on TRN2 — with fp32 PSUM accumulation):
  phase A1: W'T[k,o] = wt[k,o] + (lora_A^T @ lora_B^T)[k,o]
            (PE matmul K=16 + DVE add, 32 resident [128,512] k-tiles)
  phase B:  y[t,o] = (sum_k x^T[k,t] * W'T[k,o]) * scale[o]
            64 token-tiles x 32 k-tile accumulating matmuls into PSUM,
            DVE epilogue multiplies by scale while evicting PSUM->SBUF
  phase A2 (off critical path): norm2 = ones^T @ (W'T)^2 via PE partition
            reduction; scale = magnitude / sqrt(norm2), broadcast to 128
            partitions once via GPSIMD.

x^T is pre-packed on the host as [group, k-tile, 128, 512] so every x DMA is
one linear 128KB HBM read.
"""
import numpy as np

import concourse.bass as bass
import concourse.bass_isa as bass_isa
import concourse.mybir as mybir
import concourse.tile as tile
from concourse import bacc
from concourse.bass_utils import run_bass_kernel_spmd

P = 128
f32 = mybir.dt.float32
DT = mybir.dt.float16
NP_DT = np.float16

IN_F = 4096          # contraction dim K
OUT_F = 4096
O = 512              # out-features per core
KT = IN_F // P       # 32 k-tiles
N_CORES = 8
M_TOKENS = 8192
T_GROUP = 512


def _build(m_tokens=M_TOKENS, t_group=T_GROUP):
    n_groups = m_tokens // t_group
    groups = [t_group] * n_groups

    nc = bacc.Bacc(None, target_bir_lowering=False, debug=False)

    xt_d = nc.dram_tensor(
        "xt", [n_groups, KT, P, t_group], DT, kind="ExternalInput"
    )
    wt_d = nc.dram_tensor("wt", [IN_F, O], DT, kind="ExternalInput")
    a_d = nc.dram_tensor("a", [16, IN_F], DT, kind="ExternalInput")
    bt_d = nc.dram_tensor("bt", [16, O], DT, kind="ExternalInput")
    mg_d = nc.dram_tensor("mg", [1, O], f32, kind="ExternalInput")
    ones_d = nc.dram_tensor("ones", [P, 1], DT, kind="ExternalInput")
    y_d = nc.dram_tensor("y", [m_tokens, O], f32, kind="ExternalOutput")

    with tile.TileContext(nc) as tc:
        with (
            tc.tile_pool(name="const", bufs=1) as const_pool,
            tc.tile_pool(name="wpt", bufs=KT) as wpt_pool,
            tc.tile_pool(name="wtin", bufs=3) as wtin_pool,
            tc.tile_pool(name="sq", bufs=KT) as sq_pool,
            tc.tile_pool(name="xg", bufs=2 * KT) as xg_pool,
            tc.tile_pool(name="yout", bufs=8) as y_pool,
            tc.tile_pool(name="ps_ba", bufs=1, space="PSUM") as ps_ba_pool,
            tc.tile_pool(name="ps_y", bufs=7, space="PSUM") as ps_y_pool,
        ):
            a_sb = const_pool.tile([16, IN_F], DT, name="a_sb")
            bt_sb = const_pool.tile([16, O], DT, name="bt_sb")
            m_sb = const_pool.tile([1, O], f32, name="m_sb")
            ones_sb = const_pool.tile([P, 1], DT, name="ones_sb")
            nc.sync.dma_start(a_sb[:], a_d[:])
            nc.sync.dma_start(bt_sb[:], bt_d[:])
            nc.sync.dma_start(m_sb[:], mg_d[:])
            nc.sync.dma_start(ones_sb[:], ones_d[:])

            # ---- phase A1: adapted weight tiles (gates the main GEMM) ----
            # wt and group-0 x DMAs alternate on the same queue so wpt[kt]
            # and xg0[kt] arrive in lockstep — the first token-tile's matmuls
            # stream right behind the DMA pair stream
            wpt = []
            xg0 = []
            for kt in range(KT):
                pba = ps_ba_pool.tile([P, O], f32, name="pba", tag="pba")
                nc.tensor.matmul(
                    pba[:], a_sb[:, kt * P:(kt + 1) * P], bt_sb[:],
                    start=True, stop=True,
                )
                wtin = wtin_pool.tile([P, O], DT, name="wtin", tag="wtin")
                nc.sync.dma_start(wtin[:], wt_d[kt * P:(kt + 1) * P, :])
                xt_t = xg_pool.tile([P, t_group], DT, name=f"xg{kt}", tag="xg")
                nc.sync.dma_start(xt_t[:], xt_d[0, kt])
                xg0.append(xt_t)
                wp = wpt_pool.tile([P, O], DT, name=f"wpt{kt}", tag="wpt")
                nc.vector.tensor_add(wp[:], pba[:], wtin[:])
                wpt.append(wp)

            def emit_group_mms(g_i, g_sz, xg=None):
                subs = g_sz // P
                if xg is None:
                    xg = []
                    for kt in range(KT):
                        xt_t = xg_pool.tile(
                            [P, t_group], DT, name=f"xg{kt}", tag="xg"
                        )
                        nc.sync.dma_start(xt_t[:], xt_d[g_i, kt])
                        xg.append(xt_t)
                pys = [
                    ps_y_pool.tile([P, O], f32, name="py", tag="py")
                    for _ in range(subs)
                ]
                for sub in range(subs):
                    for kt in range(KT):
                        nc.tensor.matmul(
                            pys[sub][:],
                            xg[kt][:, sub * P:(sub + 1) * P], wpt[kt][:],
                            start=(kt == 0), stop=(kt == KT - 1),
                        )
                return pys

            def emit_epilogues(t0, pys):
                for sub, py in enumerate(pys):
                    yt = y_pool.tile([P, O], f32, name="yt", tag="yt")
                    nc.vector.tensor_mul(yt[:], py[:], scale_t[:])
                    r0 = t0 + sub * P
                    nc.scalar.dma_start(y_d[r0:r0 + P, :], yt[:])

            # ---- phase B group 0 matmuls (PE works while norms compute) ----
            pys0 = emit_group_mms(0, groups[0], xg=xg0)

            # ---- phase A2: column norms + scale (entirely off the PE: ACT
            # squares, DVE accumulates, GPSIMD reduces across partitions;
            # only the first epilogue waits on scale_t, buffered by ps_y) ----
            sqsum = const_pool.tile([P, O], f32, name="sqsum")
            for kt in range(KT):
                sq = sq_pool.tile([P, O], f32, name="sq", tag="sq")
                nc.scalar.square(sq[:], wpt[kt][:])
                if kt == 0:
                    nc.vector.tensor_copy(sqsum[:], sq[:])
                else:
                    nc.vector.tensor_add(sqsum[:], sqsum[:], sq[:])
            nsum = const_pool.tile([P, O], f32, name="nsum")
            nc.gpsimd.partition_all_reduce(
                nsum[:], sqsum[:], channels=P, reduce_op=bass_isa.ReduceOp.add
            )
            nrm = const_pool.tile([1, O], f32, name="nrm")
            nc.scalar.sqrt(nrm[:], nsum[:1, :])
            inv = const_pool.tile([1, O], f32, name="inv")
            nc.vector.reciprocal(inv[:], nrm[:])
            srow = const_pool.tile([1, O], f32, name="srow")
            nc.vector.tensor_mul(srow[:], inv[:], m_sb[:])
            scale_t = const_pool.tile([P, O], f32, name="scale_t")
            nc.gpsimd.partition_broadcast(scale_t[:], srow[:])

            emit_epilogues(0, pys0)

            # ---- phase B: remaining groups ----
            t0 = groups[0]
            for g_i, g_sz in enumerate(groups[1:], start=1):
                pys = emit_group_mms(g_i, g_sz)
                emit_epilogues(t0, pys)
                t0 += g_sz

    nc.compile()
    return nc


_NC_CACHE = {}


def kernel(x, base_weight, lora_A, lora_B, magnitude_vec):
    x = np.asarray(x)
    base_weight = np.asarray(base_weight)
    lora_A = np.asarray(lora_A)
    lora_B = np.asarray(lora_B)
    magnitude_vec = np.asarray(magnitude_vec)

    out_shape = (*x.shape[:-1], base_weight.shape[0])
    m_tokens = x.size // IN_F

    if m_tokens not in _NC_CACHE:
        _NC_CACHE[m_tokens] = _build(m_tokens)
    nc = _NC_CACHE[m_tokens]

    n_groups = m_tokens // T_GROUP
    xt = np.ascontiguousarray(x.reshape(m_tokens, IN_F).T).astype(NP_DT)
    xt = np.ascontiguousarray(
        xt.reshape(KT, P, n_groups, T_GROUP).transpose(2, 0, 1, 3)
    )
    a_c = lora_A.astype(NP_DT)
    b_c = lora_B.astype(NP_DT)
    ones = np.ones((P, 1), dtype=NP_DT)

    in_maps = []
    for c in range(N_CORES):
        sl = slice(c * O, (c + 1) * O)
        in_maps.append({
            "xt": xt,
            "wt": np.ascontiguousarray(base_weight[sl].T).astype(NP_DT),
            "a": a_c,
            "bt": np.ascontiguousarray(b_c[sl].T),
            "mg": np.ascontiguousarray(
                magnitude_vec[sl][None, :].astype(np.float32)
            ),
            "ones": ones,
        })

    res = run_bass_kernel_spmd(nc, in_maps, core_ids=list(range(N_CORES)))
    y = np.concatenate(
        [res.results[c]["y"] for c in range(N_CORES)], axis=1
    )
    return y.reshape(out_shape).astype(np.float32)
